# revision 38
# baseline (speedup 1.0000x reference)
"""Discrete Hawkes conditional-intensity kernel for 8 Trainium2 NeuronCores.

Math
----
Reference computes, per query i with (t, s) = (t_i, s_i):

    lam_i = clip(mu[s] + alpha[s, s] * b * F[t, s], 1e-5)
    F[t, s] = sum_{tp < t} obs[tp, s] * exp(-b * (t - tp))

F obeys F[t+1] = e * (F[t] + obs[t]), e = exp(-b): an exponentially-decayed
prefix sum over time.  Each core builds the table
G[t, sl] = mu[sl] + alpha[sl,sl]*b*F[t, sl] for its 32 s-columns with a
blocked formulation (time blocks of 128 on the PE array + a 32-step
cross-block carry), kept in SBUF as [128 (t&127), 1024 x=(t>>7)*32+sl] bf16.

Sharding: by SPACE.  Core c owns s-columns [32c, 32c+32) (the time scan is
core-local, so no collectives).  Queries are routed host-side to the core
owning their s value.

Gather: done entirely on the PE/DVE (gpsimd ucode-library gathers pay a
fixed ~40us Q7 IRAM load; per-element indirect DMA pays ~1us per 128
elements -- both measured dead ends).  Queries are routed host-side into 8
x-chunks (chunk = x>>7); for each chunk a one-hot-of-(t&127) matmul selects
each query's 128-wide G row slice (PE does 128-way selection at one column
per cycle), a host-supplied one-hot-of-(x&127) mask isolates the wanted
element (DVE), and a ones-vector matmul compresses the 128 rows to the
final value.  All one-hots are host-prepared routing metadata.
"""

import os
import sys

import numpy as np

_REPO_CANDIDATES = ("/opt/trn_rl_repo", os.path.expanduser("~/.axon_site/_ro/trn_rl_repo"))
for _p in _REPO_CANDIDATES:
    if os.path.isdir(_p) and _p not in sys.path:
        sys.path.append(_p)

import ml_dtypes
import concourse.bass as bass
import concourse.tile as tile
from concourse import bacc, mybir
from concourse.bass_utils import run_bass_kernel_spmd

# Problem constants (hardcoded per spec).
N_TIME = 4096
N_SPACE = 256
BATCH = 65536
N_CORES = 8
LAM_MIN = 1e-5

P = 128                  # partitions / time-block size
J = N_TIME // P          # 32 time blocks
SL = N_SPACE // N_CORES  # 32 s-columns per core
XW = J * SL              # 1024 free elements of the G table per partition
CHK = 8                  # x-chunks (x>>7)
NIC = 1152               # query slots per chunk (mean 1024, max seen 1104)
NSLOT = CHK * NIC        # device output length
SUBS = ((0, 512), (512, 1024), (1024, NIC))  # PSUM-bank-sized column splits

f32 = mybir.dt.float32
bf16 = mybir.dt.bfloat16
i32 = mybir.dt.int32
i16 = mybir.dt.int16
i8 = mybir.dt.int8
Alu = mybir.AluOpType
Act = mybir.ActivationFunctionType


def build_nc():
    nc = bacc.Bacc("TRN2", target_bir_lowering=False, debug=False)

    obs_h = nc.dram_tensor("obsr", [P, XW], i8, kind="ExternalInput")
    ohp_h = nc.dram_tensor("ohp", [P, NSLOT], bf16, kind="ExternalInput")
    msk_h = nc.dram_tensor("msk", [P, NSLOT], bf16, kind="ExternalInput")
    mus_h = nc.dram_tensor("mus", [SL], f32, kind="ExternalInput")
    ads_h = nc.dram_tensor("ads", [SL], f32, kind="ExternalInput")
    beta_h = nc.dram_tensor("beta", [1], f32, kind="ExternalInput")
    # static integer ramps (host constants; gpsimd iota would drag in a
    # ucode library swap)
    xd_h = nc.dram_tensor("xd", [P, P], i8, kind="ExternalInput")    # f - p
    xv_h = nc.dram_tensor("xv", [P, 1], i8, kind="ExternalInput")    # p - 128
    xc_h = nc.dram_tensor("xc", [J, J], i8, kind="ExternalInput")    # f - 1 - p
    xu_h = nc.dram_tensor("xu", [2, P], i8, kind="ExternalInput")    # f
    out_h = nc.dram_tensor("out", [NSLOT], f32, kind="ExternalOutput")

    from contextlib import ExitStack

    with tile.TileContext(nc) as tc, ExitStack() as ctx:
        sb = ctx.enter_context(tc.tile_pool(name="sb", bufs=1))
        psA = ctx.enter_context(tc.tile_pool(name="psA", bufs=4, space="PSUM"))
        psB = ctx.enter_context(tc.tile_pool(name="psB", bufs=2, space="PSUM"))
        ps1 = ctx.enter_context(tc.tile_pool(name="ps1", bufs=1, space="PSUM"))
        sb2 = ctx.enter_context(tc.tile_pool(name="sb2", bufs=4))

        # ---- input loads -------------------------------------------------
        # beta gates the whole decay-constant chain: land it first.
        beta_bc = sb.tile([P, 1], f32)
        nc.scalar.dma_start(beta_bc[:], bass.AP(beta_h, 0, [[0, P], [1, 1]]))
        ads_f = sb.tile([1, SL], f32)
        nc.scalar.dma_start(ads_f[:], bass.AP(ads_h, 0, [[0, 1], [1, SL]]))
        mu_f = sb.tile([1, SL], f32)
        nc.scalar.dma_start(mu_f[:], bass.AP(mus_h, 0, [[0, 1], [1, SL]]))

        obs_i = sb.tile([P, XW], i8)   # [p, (j, sl)] = obs[j*128+p, 32c+sl]
        nc.sync.dma_start(obs_i[:], obs_h.ap())

        # early routing chunks on sync; ramps + late chunks on the otherwise
        # idle gpsimd queue
        ohp = sb.tile([P, NSLOT], bf16)
        msk = sb.tile([P, NSLOT], bf16)

        def load_chunk(c, eng):
            lo, hi = c * NIC, (c + 1) * NIC
            eng.dma_start(ohp[:, lo:hi], bass.AP(ohp_h, lo, [[NSLOT, P], [1, NIC]]))
            eng.dma_start(msk[:, lo:hi], bass.AP(msk_h, lo, [[NSLOT, P], [1, NIC]]))

        for c in range(3):
            load_chunk(c, nc.sync)

        xv = sb.tile([P, 1], i8)
        nc.gpsimd.dma_start(xv[:], xv_h.ap())
        xd = sb.tile([P, P], i8)
        nc.gpsimd.dma_start(xd[:], xd_h.ap())
        xc = sb.tile([J, J], i8)
        nc.gpsimd.dma_start(xc[:], xc_h.ap())
        xu = sb.tile([2, P], i8)
        nc.gpsimd.dma_start(xu[:], xu_h.ap())
        for c in range(3, CHK):
            load_chunk(c, nc.gpsimd)

        # ---- runtime constants from beta --------------------------------
        ones1 = sb.tile([1, P], f32)
        nc.vector.memset(ones1[:], 1.0)
        for _ in range(16):
            warm0 = psA.tile([P, 512], f32, tag="pa")
            nc.tensor.matmul(warm0[:, 0:128], lhsT=ones1[:], rhs=ones1[:],
                             start=True, stop=True)
        negb = sb.tile([P, 1], f32)
        nc.vector.tensor_scalar(out=negb[:], in0=beta_bc[:], scalar1=-1.0,
                                scalar2=None, op0=Alu.mult)
        # asb[sl] = b * alpha[s, s], broadcast to 128 partitions via PE
        asb_row = sb.tile([1, SL], f32)
        nc.vector.tensor_scalar(out=asb_row[:], in0=ads_f[:],
                                scalar1=beta_bc[:1, :], scalar2=None, op0=Alu.mult)
        # v[tp] = exp(-b (128 - tp)); xv holds p - 128 so scale by +b
        vm = sb.tile([P, 1], f32)
        nc.vector.tensor_scalar(out=vm[:], in0=xv[:], scalar1=beta_bc[:],
                                scalar2=None, op0=Alu.mult)
        vv = sb.tile([P, 1], f32)
        nc.scalar.activation(vv[:], vm[:], Act.Exp)
        negb128 = sb.tile([P, 1], f32)
        nc.vector.tensor_scalar(out=negb128[:], in0=negb[:], scalar1=128.0,
                                scalar2=None, op0=Alu.mult)

        # plain bf16 copy of obs: lets the carry chain start before the
        # alpha*b scale is even available (scale folded into c32 later)
        obs_b = sb.tile([P, XW], bf16)
        nc.vector.tensor_copy(obs_b[:, 0:512], obs_i[:, 0:512])
        vvb = sb.tile([P, 1], bf16)
        nc.vector.tensor_copy(vvb[:], vv[:])
        nc.vector.tensor_copy(obs_b[:, 512:1024], obs_i[:, 512:1024])

        # r[j, sl] = sum_tp vv[tp] * obs_b[tp, j, sl]   (end-of-block sums)
        r_flat = sb.tile([1, XW], f32)
        r32 = sb.tile([J, SL], f32)
        for h in range(2):
            r_ps = psB.tile([1, 512], f32, tag="pb")
            nc.tensor.matmul(r_ps[:], lhsT=vvb[:],
                             rhs=obs_b[:, h * 512:(h + 1) * 512],
                             start=True, stop=True)
            if h == 0:
                nc.vector.tensor_copy(r_flat[:, 0:512], r_ps[:])
            else:
                nc.scalar.activation(r_flat[:, 512:1024], r_ps[:], Act.Copy)
            nc.scalar.dma_start(r32[h * 16:(h + 1) * 16, :],
                                r_flat[:, h * 512:(h + 1) * 512])

        # LcT[k, j] = exp(-128 b (j - 1 - k)) for k <= j-1 else 0
        lca = sb.tile([J, J], f32)
        nc.vector.tensor_scalar(out=lca[:], in0=xc[:], scalar1=negb128[:J, :],
                                scalar2=None, op0=Alu.mult)
        lcb = sb.tile([J, J], f32)
        nc.vector.tensor_scalar(out=lcb[:], in0=xc[:], scalar1=1000.0,
                                scalar2=None, op0=Alu.mult)
        lcm = sb.tile([J, J], f32)
        nc.vector.tensor_tensor(out=lcm[:], in0=lca[:], in1=lcb[:], op=Alu.min)
        lct = sb.tile([J, J], f32)
        nc.scalar.activation(lct[:], lcm[:], Act.Exp)

        asb_ps = ps1.tile([P, SL], f32)
        nc.tensor.matmul(asb_ps[:], lhsT=ones1[:], rhs=asb_row[:], start=True, stop=True)
        asb_bc = sb.tile([P, SL], f32)
        nc.vector.tensor_copy(asb_bc[:], asb_ps[:])

        # obs_f[tp, j, sl] = obs * asb[sl]   (convert + scale, 2 halves)
        obs_f = sb.tile([P, XW], bf16)
        obs_f3 = obs_f[:].rearrange("p (j s) -> p j s", s=SL)
        obs_i3 = obs_i[:].rearrange("p (j s) -> p j s", s=SL)
        HB = J // 2
        for h in range(2):
            nc.vector.tensor_tensor(
                out=obs_f3[:, h * HB:(h + 1) * HB, :],
                in0=obs_i3[:, h * HB:(h + 1) * HB, :],
                in1=asb_bc[:].unsqueeze(1).broadcast_to((P, HB, SL)),
                op=Alu.mult,
            )

        # LdT[tp, m] = exp(-b (m - tp)) for tp < m else 0   (within-block decay)
        lda = sb.tile([P, P], f32)
        nc.vector.tensor_scalar(out=lda[:], in0=xd[:], scalar1=negb[:],
                                scalar2=None, op0=Alu.mult)
        ldb = sb.tile([P, P], f32)
        nc.vector.tensor_scalar(out=ldb[:], in0=xd[:], scalar1=1000.0,
                                scalar2=-1000.0, op0=Alu.mult, op1=Alu.add)
        ldm = sb.tile([P, P], f32)
        nc.vector.tensor_tensor(out=ldm[:], in0=lda[:], in1=ldb[:], op=Alu.min)
        ldt = sb.tile([P, P], f32)
        nc.scalar.activation(ldt[:], ldm[:], Act.Exp)
        ldtb = sb.tile([P, P], bf16)
        nc.vector.tensor_copy(ldtb[:], ldt[:])

        # u2: row0 = exp(-b i), row1 = ones (mu term).
        negb01 = sb.tile([2, 1], f32)
        nc.vector.memset(negb01[:], 0.0)
        nc.vector.tensor_copy(negb01[0:1, :], negb[0:1, :])
        um = sb.tile([2, P], f32)
        nc.vector.tensor_scalar(out=um[:], in0=xu[:], scalar1=negb01[:],
                                scalar2=None, op0=Alu.mult)
        u2 = sb.tile([2, P], f32)
        nc.scalar.activation(u2[:], um[:], Act.Exp)
        u2b = sb.tile([2, P], bf16)
        nc.vector.tensor_copy(u2b[:], u2[:])

        # column of ones for the final 128-row compress
        onec = sb.tile([P, 1], bf16)
        nc.vector.memset(onec[:], 1.0)

        # ---- carry combine (scaled by alpha*b here), split in halves so
        # the j<16 carries (needing only the first half of r) unblock the
        # first half of G early -----------------------------------------
        rhs2 = sb.tile([2, XW], bf16)  # row0 = carry C flat, row1 = mu tiled
        mu_b = sb.tile([1, SL], bf16)
        nc.vector.tensor_copy(mu_b[:], mu_f[:])
        nc.scalar.dma_start(
            rhs2[1:2, :].rearrange("o (j s) -> o j s", s=SL),
            mu_b[:].unsqueeze(1).broadcast_to((1, J, SL)))

        g_sb = sb.tile([P, XW], bf16)
        HJ = J // 2
        for h in range(2):
            c_ps = ps1.tile([HJ, SL], f32, tag="cps")
            nc.tensor.matmul(c_ps[:], lhsT=lct[0:(h + 1) * HJ, h * HJ:(h + 1) * HJ],
                             rhs=r32[0:(h + 1) * HJ, :], start=True, stop=True)
            c32 = sb2.tile([HJ, SL], bf16, tag=f"c32{h}")
            nc.vector.tensor_tensor(out=c32[:], in0=c_ps[:], in1=asb_bc[0:HJ, :],
                                    op=Alu.mult)
            nc.scalar.dma_start(rhs2[0:1, h * 512:(h + 1) * 512], c32[:])

            pch = psA.tile([P, 512], f32, tag="pa")
            nc.tensor.matmul(pch[:], lhsT=ldtb[:],
                             rhs=obs_f[:, h * 512:(h + 1) * 512],
                             start=True, stop=True)
            nc.tensor.matmul(pch[:], lhsT=u2b[:],
                             rhs=rhs2[:, h * 512:(h + 1) * 512],
                             start=False, stop=True, skip_group_check=True)
            if h == 0:
                nc.vector.tensor_copy(g_sb[:, 0:512], pch[:])
            else:
                nc.scalar.activation(g_sb[:, 512:1024], pch[:], Act.Copy)

        # PE p-state warmup: dependency-free matmuls fill the otherwise idle
        # window while the carry chain drains, keeping the clock at full
        # speed for the G build and first gather chunks.
        for _ in range(18):
            warm = psB.tile([1, 512], f32, tag="pb")
            nc.tensor.matmul(warm[:, 0:256], lhsT=vvb[:], rhs=obs_f[:, 0:256],
                             start=True, stop=True)

        # ---- chunked select / mask / compress ---------------------------
        lam = sb.tile([1, NSLOT], f32)

        def stage1(c):
            outs = []
            for (lo, hi) in SUBS:
                w = hi - lo
                pa = psA.tile([P, 512], f32, tag="pa")
                nc.tensor.matmul(pa[:, 0:w], lhsT=g_sb[:, 128 * c:128 * (c + 1)],
                                 rhs=ohp[:, c * NIC + lo:c * NIC + hi],
                                 start=True, stop=True)
                outs.append(pa)
            return outs

        def stage2(c, pas):
            for k, (lo, hi) in enumerate(SUBS):
                w = hi - lo
                mskd = sb2.tile([P, 512], bf16, tag="mskd")
                nc.vector.tensor_tensor(
                    out=mskd[:, 0:w], in0=pas[k][:, 0:w],
                    in1=msk[:, c * NIC + lo:c * NIC + hi], op=Alu.mult)
                pb = psB.tile([1, 512], f32, tag="pb")
                nc.tensor.matmul(pb[:, 0:w], lhsT=onec[:], rhs=mskd[:, 0:w],
                                 start=True, stop=True)
                # evict off the PE's critical path; +LAM_MIN stands in for
                # the clip (lam >= 0 always, values far above LAM_MIN, so the
                # additive shift is ~0.3% worst-case -- well inside tol).
                # k=0,1 on ACT, the small k=2 on DVE to keep ACT from
                # straggling at the end of the chunk pipeline.
                if k < 2:
                    nc.scalar.activation(
                        lam[:, c * NIC + lo:c * NIC + hi], pb[:, 0:w],
                        Act.Copy, bias=float(LAM_MIN))
                else:
                    nc.vector.tensor_scalar(
                        out=lam[:, c * NIC + lo:c * NIC + hi], in0=pb[:, 0:w],
                        scalar1=float(LAM_MIN), scalar2=None, op0=Alu.add)

        prev = None
        for c in range(CHK):
            pas = stage1(c)
            if prev is not None:
                stage2(c - 1, prev)
            prev = pas
        stage2(CHK - 1, prev)

        half = (CHK // 2) * NIC
        nc.scalar.dma_start(bass.AP(out_h, 0, [[0, 1], [1, half]]),
                            lam[:, 0:half])
        nc.scalar.dma_start(bass.AP(out_h, half, [[0, 1], [1, NSLOT - half]]),
                            lam[:, half:NSLOT])

    nc.compile()
    return nc


_NC_CACHE = None


def _get_nc():
    global _NC_CACHE
    if _NC_CACHE is None:
        _NC_CACHE = build_nc()
    return _NC_CACHE


def prepare_in_maps(t, s, obs, mu, alpha, beta):
    """Route queries to cores by s-range; build per-core device inputs.

    Returns (in_maps, perms); perms[c] = (dev_pos, orig_pos) with
    out[orig_pos] = dev_out[dev_pos].
    """
    t = np.ascontiguousarray(np.asarray(t, dtype=np.int32))
    s = np.ascontiguousarray(np.asarray(s, dtype=np.int32))
    obs = np.asarray(obs)
    mu = np.asarray(mu, dtype=np.float32)
    alpha = np.asarray(alpha, dtype=np.float32)
    beta = np.ascontiguousarray(np.asarray(beta, dtype=np.float32))
    adiag = np.ascontiguousarray(np.diagonal(alpha)).astype(np.float32)
    obs8 = obs.astype(np.int8)  # values in [0, 10)

    pp = np.arange(P, dtype=np.int32)
    ff = np.arange(P, dtype=np.int32)
    xd = (ff[None, :] - pp[:, None]).astype(np.int8)                 # f - p
    xv = (pp[:, None] - P).astype(np.int8)                           # p - 128
    kk = np.arange(J, dtype=np.int32)
    xc = (kk[None, :] - 1 - kk[:, None]).astype(np.int8)             # f - 1 - p
    xu = np.broadcast_to(ff[None, :], (2, P)).astype(np.int8).copy() # f

    in_maps, perms = [], []
    for c in range(N_CORES):
        sel = np.nonzero((s >> 5) == c)[0]
        tc_, sc_ = t[sel], s[sel]
        x = (tc_ >> 7) * SL + (sc_ & (SL - 1))   # G free index, [0, 1024)
        chunk = x >> 7                            # [0, 8)
        ohp = np.zeros((P, NSLOT), np.float32)
        msk = np.zeros((P, NSLOT), np.float32)
        dev_pos = np.empty(len(sel), np.int64)
        for cc in range(CHK):
            qs = np.nonzero(chunk == cc)[0]
            n = len(qs)
            if n > NIC:
                raise RuntimeError(f"core {c} chunk {cc}: {n} queries > {NIC}")
            i = np.arange(n)
            col = cc * NIC + i
            ohp[tc_[qs] & (P - 1), col] = 1.0
            msk[x[qs] & (P - 1), col] = 1.0
            dev_pos[qs] = col
        obsr = np.ascontiguousarray(
            obs8[:, c * SL:(c + 1) * SL]
            .reshape(J, P, SL).transpose(1, 0, 2).reshape(P, XW))
        in_maps.append({
            "obsr": obsr,
            "ohp": ohp.astype(ml_dtypes.bfloat16),
            "msk": msk.astype(ml_dtypes.bfloat16),
            "mus": np.ascontiguousarray(mu[c * SL:(c + 1) * SL]),
            "ads": np.ascontiguousarray(adiag[c * SL:(c + 1) * SL]),
            "beta": beta,
            "xd": xd, "xv": xv, "xc": xc, "xu": xu,
        })
        perms.append((dev_pos, sel))
    return in_maps, perms


def finalize(results, perms):
    out = np.empty(BATCH, np.float32)
    for c in range(N_CORES):
        dev = results[c]["out"]
        dev_pos, orig_pos = perms[c]
        out[orig_pos] = dev[dev_pos]
    return out


def kernel(t, s, obs, mu, alpha, beta, **_unused):
    nc = _get_nc()
    in_maps, perms = prepare_in_maps(t, s, obs, mu, alpha, beta)
    res = run_bass_kernel_spmd(nc, in_maps, core_ids=list(range(N_CORES)))
    return finalize(res.results, perms)


if __name__ == "__main__":
    # quick self-check against a numpy re-implementation on random data
    rng = np.random.default_rng(0)
    t = rng.integers(0, N_TIME, BATCH).astype(np.int32)
    s = rng.integers(0, N_SPACE, BATCH).astype(np.int32)
    obs = rng.integers(0, 10, (N_TIME, N_SPACE)).astype(np.int32)
    mu = rng.random(N_SPACE, dtype=np.float32)
    alpha = rng.random((N_SPACE, N_SPACE), dtype=np.float32)
    beta = (rng.random(1, dtype=np.float32) + 0.1).astype(np.float32)

    got = kernel(t=t, s=s, obs=obs, mu=mu, alpha=alpha, beta=beta)

    b = float(beta[0])
    e = np.exp(-b)
    F = np.zeros((N_TIME, N_SPACE), np.float64)
    for tt in range(1, N_TIME):
        F[tt] = e * (F[tt - 1] + obs[tt - 1])
    G = np.clip(mu[None, :] + np.diag(alpha)[None, :] * b * F, LAM_MIN, None)
    want = G[t, s].astype(np.float32)
    err = np.abs(got - want) / np.maximum(np.abs(want), 1e-6)
    print("max rel err:", err.max(), "mean:", err.mean())


# revision 39
# speedup vs baseline: 1.0200x; 1.0200x over previous
"""Discrete Hawkes conditional-intensity kernel for 8 Trainium2 NeuronCores.

Math
----
Reference computes, per query i with (t, s) = (t_i, s_i):

    lam_i = clip(mu[s] + alpha[s, s] * b * F[t, s], 1e-5)
    F[t, s] = sum_{tp < t} obs[tp, s] * exp(-b * (t - tp))

F obeys F[t+1] = e * (F[t] + obs[t]), e = exp(-b): an exponentially-decayed
prefix sum over time.  Each core builds the table
G[t, sl] = mu[sl] + alpha[sl,sl]*b*F[t, sl] for its 32 s-columns with a
blocked formulation (time blocks of 128 on the PE array + a 32-step
cross-block carry), kept in SBUF as [128 (t&127), 1024 x=(t>>7)*32+sl] bf16.

Sharding: by SPACE.  Core c owns s-columns [32c, 32c+32) (the time scan is
core-local, so no collectives).  Queries are routed host-side to the core
owning their s value.

Gather: done entirely on the PE/DVE (gpsimd ucode-library gathers pay a
fixed ~40us Q7 IRAM load; per-element indirect DMA pays ~1us per 128
elements -- both measured dead ends).  Queries are routed host-side into 8
x-chunks (chunk = x>>7); for each chunk a one-hot-of-(t&127) matmul selects
each query's 128-wide G row slice (PE does 128-way selection at one column
per cycle), a host-supplied one-hot-of-(x&127) mask isolates the wanted
element (DVE), and a ones-vector matmul compresses the 128 rows to the
final value.  All one-hots are host-prepared routing metadata.
"""

import os
import sys

import numpy as np

_REPO_CANDIDATES = ("/opt/trn_rl_repo", os.path.expanduser("~/.axon_site/_ro/trn_rl_repo"))
for _p in _REPO_CANDIDATES:
    if os.path.isdir(_p) and _p not in sys.path:
        sys.path.append(_p)

import ml_dtypes
import concourse.bass as bass
import concourse.tile as tile
from concourse import bacc, mybir
from concourse.bass_utils import run_bass_kernel_spmd

# Problem constants (hardcoded per spec).
N_TIME = 4096
N_SPACE = 256
BATCH = 65536
N_CORES = 8
LAM_MIN = 1e-5

P = 128                  # partitions / time-block size
J = N_TIME // P          # 32 time blocks
SL = N_SPACE // N_CORES  # 32 s-columns per core
XW = J * SL              # 1024 free elements of the G table per partition
CHK = 8                  # x-chunks (x>>7)
NIC = 1152               # query slots per chunk (mean 1024, max seen 1104)
NSLOT = CHK * NIC        # device output length
SUBS = ((0, 512), (512, 1024), (1024, NIC))  # PSUM-bank-sized column splits

f32 = mybir.dt.float32
bf16 = mybir.dt.bfloat16
i32 = mybir.dt.int32
i16 = mybir.dt.int16
i8 = mybir.dt.int8
Alu = mybir.AluOpType
Act = mybir.ActivationFunctionType


def build_nc():
    nc = bacc.Bacc("TRN2", target_bir_lowering=False, debug=False)

    obs_h = nc.dram_tensor("obsr", [P, XW], i8, kind="ExternalInput")
    ohp_h = nc.dram_tensor("ohp", [P, NSLOT], bf16, kind="ExternalInput")
    msk_h = nc.dram_tensor("msk", [P, NSLOT], bf16, kind="ExternalInput")
    mus_h = nc.dram_tensor("mus", [SL], f32, kind="ExternalInput")
    ads_h = nc.dram_tensor("ads", [SL], f32, kind="ExternalInput")
    beta_h = nc.dram_tensor("beta", [1], f32, kind="ExternalInput")
    # static integer ramps (host constants; gpsimd iota would drag in a
    # ucode library swap)
    xd_h = nc.dram_tensor("xd", [P, P], i8, kind="ExternalInput")    # f - p
    xv_h = nc.dram_tensor("xv", [P, 1], i8, kind="ExternalInput")    # p - 128
    xc_h = nc.dram_tensor("xc", [J, J], i8, kind="ExternalInput")    # f - 1 - p
    xu_h = nc.dram_tensor("xu", [2, P], i8, kind="ExternalInput")    # f
    out_h = nc.dram_tensor("out", [NSLOT], f32, kind="ExternalOutput")

    from contextlib import ExitStack

    with tile.TileContext(nc) as tc, ExitStack() as ctx:
        sb = ctx.enter_context(tc.tile_pool(name="sb", bufs=1))
        psA = ctx.enter_context(tc.tile_pool(name="psA", bufs=4, space="PSUM"))
        psB = ctx.enter_context(tc.tile_pool(name="psB", bufs=2, space="PSUM"))
        ps1 = ctx.enter_context(tc.tile_pool(name="ps1", bufs=1, space="PSUM"))
        sb2 = ctx.enter_context(tc.tile_pool(name="sb2", bufs=4))

        # ---- input loads -------------------------------------------------
        # beta gates the whole decay-constant chain: land it first.
        beta_bc = sb.tile([P, 1], f32)
        nc.scalar.dma_start(beta_bc[:], bass.AP(beta_h, 0, [[0, P], [1, 1]]))
        ads_f = sb.tile([1, SL], f32)
        nc.scalar.dma_start(ads_f[:], bass.AP(ads_h, 0, [[0, 1], [1, SL]]))
        mu_f = sb.tile([1, SL], f32)
        nc.scalar.dma_start(mu_f[:], bass.AP(mus_h, 0, [[0, 1], [1, SL]]))

        obs_i = sb.tile([P, XW], i8)   # [p, (j, sl)] = obs[j*128+p, 32c+sl]
        nc.sync.dma_start(obs_i[:], obs_h.ap())

        # early routing chunks on sync; ramps + late chunks on the otherwise
        # idle gpsimd queue
        ohp = sb.tile([P, NSLOT], bf16)
        msk = sb.tile([P, NSLOT], bf16)

        def load_chunk(c, eng):
            lo, hi = c * NIC, (c + 1) * NIC
            eng.dma_start(ohp[:, lo:hi], bass.AP(ohp_h, lo, [[NSLOT, P], [1, NIC]]))
            eng.dma_start(msk[:, lo:hi], bass.AP(msk_h, lo, [[NSLOT, P], [1, NIC]]))

        for c in range(3):
            load_chunk(c, nc.sync)

        xv = sb.tile([P, 1], i8)
        nc.gpsimd.dma_start(xv[:], xv_h.ap())
        xd = sb.tile([P, P], i8)
        nc.gpsimd.dma_start(xd[:], xd_h.ap())
        xc = sb.tile([J, J], i8)
        nc.gpsimd.dma_start(xc[:], xc_h.ap())
        xu = sb.tile([2, P], i8)
        nc.gpsimd.dma_start(xu[:], xu_h.ap())
        for c in range(3, CHK):
            load_chunk(c, nc.gpsimd)

        # ---- runtime constants from beta --------------------------------
        ones1 = sb.tile([1, P], f32)
        nc.vector.memset(ones1[:], 1.0)
        negb = sb.tile([P, 1], f32)
        nc.vector.tensor_scalar(out=negb[:], in0=beta_bc[:], scalar1=-1.0,
                                scalar2=None, op0=Alu.mult)
        # asb[sl] = b * alpha[s, s], broadcast to 128 partitions via PE
        asb_row = sb.tile([1, SL], f32)
        nc.vector.tensor_scalar(out=asb_row[:], in0=ads_f[:],
                                scalar1=beta_bc[:1, :], scalar2=None, op0=Alu.mult)
        # v[tp] = exp(-b (128 - tp)); xv holds p - 128 so scale by +b
        vm = sb.tile([P, 1], f32)
        nc.vector.tensor_scalar(out=vm[:], in0=xv[:], scalar1=beta_bc[:],
                                scalar2=None, op0=Alu.mult)
        vv = sb.tile([P, 1], f32)
        nc.scalar.activation(vv[:], vm[:], Act.Exp)
        negb128 = sb.tile([P, 1], f32)
        nc.vector.tensor_scalar(out=negb128[:], in0=negb[:], scalar1=128.0,
                                scalar2=None, op0=Alu.mult)

        # plain bf16 copy of obs: lets the carry chain start before the
        # alpha*b scale is even available (scale folded into c32 later)
        obs_b = sb.tile([P, XW], bf16)
        nc.vector.tensor_copy(obs_b[:, 0:512], obs_i[:, 0:512])
        vvb = sb.tile([P, 1], bf16)
        nc.vector.tensor_copy(vvb[:], vv[:])
        nc.vector.tensor_copy(obs_b[:, 512:1024], obs_i[:, 512:1024])

        # r[j, sl] = sum_tp vv[tp] * obs_b[tp, j, sl]   (end-of-block sums)
        r_flat = sb.tile([1, XW], f32)
        r32 = sb.tile([J, SL], f32)
        for h in range(2):
            r_ps = psB.tile([1, 512], f32, tag="pb")
            nc.tensor.matmul(r_ps[:], lhsT=vvb[:],
                             rhs=obs_b[:, h * 512:(h + 1) * 512],
                             start=True, stop=True)
            if h == 0:
                nc.vector.tensor_copy(r_flat[:, 0:512], r_ps[:])
            else:
                nc.scalar.activation(r_flat[:, 512:1024], r_ps[:], Act.Copy)
            nc.scalar.dma_start(r32[h * 16:(h + 1) * 16, :],
                                r_flat[:, h * 512:(h + 1) * 512])

        # LcT[k, j] = exp(-128 b (j - 1 - k)) for k <= j-1 else 0
        lca = sb.tile([J, J], f32)
        nc.vector.tensor_scalar(out=lca[:], in0=xc[:], scalar1=negb128[:J, :],
                                scalar2=None, op0=Alu.mult)
        lcb = sb.tile([J, J], f32)
        nc.vector.tensor_scalar(out=lcb[:], in0=xc[:], scalar1=1000.0,
                                scalar2=None, op0=Alu.mult)
        lcm = sb.tile([J, J], f32)
        nc.vector.tensor_tensor(out=lcm[:], in0=lca[:], in1=lcb[:], op=Alu.min)
        lct = sb.tile([J, J], f32)
        nc.scalar.activation(lct[:], lcm[:], Act.Exp)

        asb_ps = ps1.tile([P, SL], f32)
        nc.tensor.matmul(asb_ps[:], lhsT=ones1[:], rhs=asb_row[:], start=True, stop=True)
        asb_bc = sb.tile([P, SL], f32)
        nc.vector.tensor_copy(asb_bc[:], asb_ps[:])

        # obs_f[tp, j, sl] = obs * asb[sl]   (convert + scale, 2 halves)
        obs_f = sb.tile([P, XW], bf16)
        obs_f3 = obs_f[:].rearrange("p (j s) -> p j s", s=SL)
        obs_i3 = obs_i[:].rearrange("p (j s) -> p j s", s=SL)
        HB = J // 2
        for h in range(2):
            nc.vector.tensor_tensor(
                out=obs_f3[:, h * HB:(h + 1) * HB, :],
                in0=obs_i3[:, h * HB:(h + 1) * HB, :],
                in1=asb_bc[:].unsqueeze(1).broadcast_to((P, HB, SL)),
                op=Alu.mult,
            )

        # LdT[tp, m] = exp(-b (m - tp)) for tp < m else 0   (within-block decay)
        lda = sb.tile([P, P], f32)
        nc.vector.tensor_scalar(out=lda[:], in0=xd[:], scalar1=negb[:],
                                scalar2=None, op0=Alu.mult)
        ldb = sb.tile([P, P], f32)
        nc.vector.tensor_scalar(out=ldb[:], in0=xd[:], scalar1=1000.0,
                                scalar2=-1000.0, op0=Alu.mult, op1=Alu.add)
        ldm = sb.tile([P, P], f32)
        nc.vector.tensor_tensor(out=ldm[:], in0=lda[:], in1=ldb[:], op=Alu.min)
        ldt = sb.tile([P, P], f32)
        nc.scalar.activation(ldt[:], ldm[:], Act.Exp)
        ldtb = sb.tile([P, P], bf16)
        nc.vector.tensor_copy(ldtb[:], ldt[:])

        # u2: row0 = exp(-b i), row1 = ones (mu term).
        negb01 = sb.tile([2, 1], f32)
        nc.vector.memset(negb01[:], 0.0)
        nc.vector.tensor_copy(negb01[0:1, :], negb[0:1, :])
        um = sb.tile([2, P], f32)
        nc.vector.tensor_scalar(out=um[:], in0=xu[:], scalar1=negb01[:],
                                scalar2=None, op0=Alu.mult)
        u2 = sb.tile([2, P], f32)
        nc.scalar.activation(u2[:], um[:], Act.Exp)
        u2b = sb.tile([2, P], bf16)
        nc.vector.tensor_copy(u2b[:], u2[:])

        # column of ones for the final 128-row compress
        onec = sb.tile([P, 1], bf16)
        nc.vector.memset(onec[:], 1.0)

        # ---- carry combine (scaled by alpha*b here), split in halves so
        # the j<16 carries (needing only the first half of r) unblock the
        # first half of G early -----------------------------------------
        rhs2 = sb.tile([2, XW], bf16)  # row0 = carry C flat, row1 = mu tiled
        mu_b = sb.tile([1, SL], bf16)
        nc.vector.tensor_copy(mu_b[:], mu_f[:])
        nc.scalar.dma_start(
            rhs2[1:2, :].rearrange("o (j s) -> o j s", s=SL),
            mu_b[:].unsqueeze(1).broadcast_to((1, J, SL)))

        g_sb = sb.tile([P, XW], bf16)
        HJ = J // 2
        for h in range(2):
            c_ps = ps1.tile([HJ, SL], f32, tag="cps")
            nc.tensor.matmul(c_ps[:], lhsT=lct[0:(h + 1) * HJ, h * HJ:(h + 1) * HJ],
                             rhs=r32[0:(h + 1) * HJ, :], start=True, stop=True)
            c32 = sb2.tile([HJ, SL], bf16, tag=f"c32{h}")
            nc.vector.tensor_tensor(out=c32[:], in0=c_ps[:], in1=asb_bc[0:HJ, :],
                                    op=Alu.mult)
            nc.scalar.dma_start(rhs2[0:1, h * 512:(h + 1) * 512], c32[:])

            pch = psA.tile([P, 512], f32, tag="pa")
            nc.tensor.matmul(pch[:], lhsT=ldtb[:],
                             rhs=obs_f[:, h * 512:(h + 1) * 512],
                             start=True, stop=True)
            nc.tensor.matmul(pch[:], lhsT=u2b[:],
                             rhs=rhs2[:, h * 512:(h + 1) * 512],
                             start=False, stop=True, skip_group_check=True)
            if h == 0:
                nc.vector.tensor_copy(g_sb[:, 0:512], pch[:])
            else:
                nc.scalar.activation(g_sb[:, 512:1024], pch[:], Act.Copy)

        # PE p-state warmup: dependency-free matmuls fill the otherwise idle
        # window while the carry chain drains, keeping the clock at full
        # speed for the G build and first gather chunks.
        for _ in range(14):
            warm = psB.tile([1, 512], f32, tag="pb")
            nc.tensor.matmul(warm[:, 0:256], lhsT=vvb[:], rhs=obs_f[:, 0:256],
                             start=True, stop=True)

        # ---- chunked select / mask / compress ---------------------------
        lam = sb.tile([1, NSLOT], f32)

        def stage1(c):
            outs = []
            for (lo, hi) in SUBS:
                w = hi - lo
                pa = psA.tile([P, 512], f32, tag="pa")
                nc.tensor.matmul(pa[:, 0:w], lhsT=g_sb[:, 128 * c:128 * (c + 1)],
                                 rhs=ohp[:, c * NIC + lo:c * NIC + hi],
                                 start=True, stop=True)
                outs.append(pa)
            return outs

        def stage2(c, pas):
            for k, (lo, hi) in enumerate(SUBS):
                w = hi - lo
                mskd = sb2.tile([P, 512], bf16, tag="mskd")
                nc.vector.tensor_tensor(
                    out=mskd[:, 0:w], in0=pas[k][:, 0:w],
                    in1=msk[:, c * NIC + lo:c * NIC + hi], op=Alu.mult)
                pb = psB.tile([1, 512], f32, tag="pb")
                nc.tensor.matmul(pb[:, 0:w], lhsT=onec[:], rhs=mskd[:, 0:w],
                                 start=True, stop=True)
                # evict on the (idle) ACT engine; +LAM_MIN stands in for the
                # clip (lam >= 0 always, and values sit far above LAM_MIN, so
                # the additive shift is ~0.3% worst-case -- well inside tol)
                nc.scalar.activation(
                    lam[:, c * NIC + lo:c * NIC + hi], pb[:, 0:w],
                    Act.Copy, bias=float(LAM_MIN))

        prev = None
        for c in range(CHK):
            pas = stage1(c)
            if prev is not None:
                stage2(c - 1, prev)
            prev = pas
        stage2(CHK - 1, prev)

        half = (CHK // 2) * NIC
        nc.scalar.dma_start(bass.AP(out_h, 0, [[0, 1], [1, half]]),
                            lam[:, 0:half])
        nc.scalar.dma_start(bass.AP(out_h, half, [[0, 1], [1, NSLOT - half]]),
                            lam[:, half:NSLOT])

    nc.compile()
    return nc


_NC_CACHE = None


def _get_nc():
    global _NC_CACHE
    if _NC_CACHE is None:
        _NC_CACHE = build_nc()
    return _NC_CACHE


def prepare_in_maps(t, s, obs, mu, alpha, beta):
    """Route queries to cores by s-range; build per-core device inputs.

    Returns (in_maps, perms); perms[c] = (dev_pos, orig_pos) with
    out[orig_pos] = dev_out[dev_pos].
    """
    t = np.ascontiguousarray(np.asarray(t, dtype=np.int32))
    s = np.ascontiguousarray(np.asarray(s, dtype=np.int32))
    obs = np.asarray(obs)
    mu = np.asarray(mu, dtype=np.float32)
    alpha = np.asarray(alpha, dtype=np.float32)
    beta = np.ascontiguousarray(np.asarray(beta, dtype=np.float32))
    adiag = np.ascontiguousarray(np.diagonal(alpha)).astype(np.float32)
    obs8 = obs.astype(np.int8)  # values in [0, 10)

    pp = np.arange(P, dtype=np.int32)
    ff = np.arange(P, dtype=np.int32)
    xd = (ff[None, :] - pp[:, None]).astype(np.int8)                 # f - p
    xv = (pp[:, None] - P).astype(np.int8)                           # p - 128
    kk = np.arange(J, dtype=np.int32)
    xc = (kk[None, :] - 1 - kk[:, None]).astype(np.int8)             # f - 1 - p
    xu = np.broadcast_to(ff[None, :], (2, P)).astype(np.int8).copy() # f

    in_maps, perms = [], []
    for c in range(N_CORES):
        sel = np.nonzero((s >> 5) == c)[0]
        tc_, sc_ = t[sel], s[sel]
        x = (tc_ >> 7) * SL + (sc_ & (SL - 1))   # G free index, [0, 1024)
        chunk = x >> 7                            # [0, 8)
        ohp = np.zeros((P, NSLOT), np.float32)
        msk = np.zeros((P, NSLOT), np.float32)
        dev_pos = np.empty(len(sel), np.int64)
        for cc in range(CHK):
            qs = np.nonzero(chunk == cc)[0]
            n = len(qs)
            if n > NIC:
                raise RuntimeError(f"core {c} chunk {cc}: {n} queries > {NIC}")
            i = np.arange(n)
            col = cc * NIC + i
            ohp[tc_[qs] & (P - 1), col] = 1.0
            msk[x[qs] & (P - 1), col] = 1.0
            dev_pos[qs] = col
        obsr = np.ascontiguousarray(
            obs8[:, c * SL:(c + 1) * SL]
            .reshape(J, P, SL).transpose(1, 0, 2).reshape(P, XW))
        in_maps.append({
            "obsr": obsr,
            "ohp": ohp.astype(ml_dtypes.bfloat16),
            "msk": msk.astype(ml_dtypes.bfloat16),
            "mus": np.ascontiguousarray(mu[c * SL:(c + 1) * SL]),
            "ads": np.ascontiguousarray(adiag[c * SL:(c + 1) * SL]),
            "beta": beta,
            "xd": xd, "xv": xv, "xc": xc, "xu": xu,
        })
        perms.append((dev_pos, sel))
    return in_maps, perms


def finalize(results, perms):
    out = np.empty(BATCH, np.float32)
    for c in range(N_CORES):
        dev = results[c]["out"]
        dev_pos, orig_pos = perms[c]
        out[orig_pos] = dev[dev_pos]
    return out


def kernel(t, s, obs, mu, alpha, beta, **_unused):
    nc = _get_nc()
    in_maps, perms = prepare_in_maps(t, s, obs, mu, alpha, beta)
    res = run_bass_kernel_spmd(nc, in_maps, core_ids=list(range(N_CORES)))
    return finalize(res.results, perms)


if __name__ == "__main__":
    # quick self-check against a numpy re-implementation on random data
    rng = np.random.default_rng(0)
    t = rng.integers(0, N_TIME, BATCH).astype(np.int32)
    s = rng.integers(0, N_SPACE, BATCH).astype(np.int32)
    obs = rng.integers(0, 10, (N_TIME, N_SPACE)).astype(np.int32)
    mu = rng.random(N_SPACE, dtype=np.float32)
    alpha = rng.random((N_SPACE, N_SPACE), dtype=np.float32)
    beta = (rng.random(1, dtype=np.float32) + 0.1).astype(np.float32)

    got = kernel(t=t, s=s, obs=obs, mu=mu, alpha=alpha, beta=beta)

    b = float(beta[0])
    e = np.exp(-b)
    F = np.zeros((N_TIME, N_SPACE), np.float64)
    for tt in range(1, N_TIME):
        F[tt] = e * (F[tt - 1] + obs[tt - 1])
    G = np.clip(mu[None, :] + np.diag(alpha)[None, :] * b * F, LAM_MIN, None)
    want = G[t, s].astype(np.float32)
    err = np.abs(got - want) / np.maximum(np.abs(want), 1e-6)
    print("max rel err:", err.max(), "mean:", err.mean())
